# revision 76
# baseline (speedup 1.0000x reference)
"""Causal self-attention on 8 Trainium2 NeuronCores — v5.

Sharding: tensor-parallel over heads through QKV+attention (2 heads/core).
Output ownership: batches 0-1 contiguous [j*256,(j+1)*256) per core;
batches 2-3 split-half (core j owns 128 tokens of each 1024-token half) so
their AllToAlls split into half-collectives; the one exposed collective at
the tail is the smallest and is covered by deferred projection work.

Pipeline: QKV is loaded just-in-time (att(b,qb) only needs token tiles up
to 4b+qb — causal), one window ahead; attention q-blocks interleave one
QKV tile or proj chunk each; per-batch AllToAlls overlap the next batch.
Softmax skips max-subtraction (logits O(+-10)). Layouts are transposed
throughout ([dim, token]); an all-ones column appended to V yields the
softmax denominator in PSUM row 64. QK matmuls are per-tile causally
narrowed (each diagonal tile only computes its valid columns).

Normalization (no DRAM bounce): DVE reciprocal of the denominator row ->
gpsimd partition_broadcast across 64 partitions -> one DVE multiply per
head into a combined bf16 scatter tile; one merged scatter DMA per
(batch, q-block). Batches 0-1 evacuate Y first (early PSUM release) and
defer the rest down the priority heap; batches 2-3 multiply straight out
of PSUM (shortest chain into their collectives).

Scheduler discipline (the Tile list scheduler reorders freely):
 - readbacks are WAR-gated behind late tiles so their semaphore waits can
   never head-of-line-block the gpsimd queue (partition_broadcasts);
 - deferred proj streams are RAW-gated behind the last q-block's exp tile
   (an exact *1.0 rewrite of one yb element) so mid-kernel bubbles cannot
   steal the tail's collective-covering work;
 - proj(3) halves sit at the bottom of the priority heap;
 - proj output stores are batched 4 blocks per DMA (per-DMA issue and
   HWDGE slots are serialized and would throttle the tail) and written in
   bf16 (+2e-4 rel err, halves store bytes); assemble() casts back to f32.
"""

import os

import numpy as np
import ml_dtypes

# Problem dims (nn_CausalSelfAttention: B=4, T=2048, D=1024, H=16)
CFG_FULL = dict(B=4, T=2048, D=1024, H=16)
NCORES = 8
KB = 128  # key tile (partition dim of probs)


def _derived(cfg):
    B, T, D, H = cfg["B"], cfg["T"], cfg["D"], cfg["H"]
    HD = D // H
    assert HD == 64, "design assumes head_dim == 64 (2 heads per 128 partitions)"
    assert H // NCORES == 2, "design assumes 2 heads per core"
    TPB = B * T
    CHUNK = TPB // NCORES   # tokens owned per core (B slices of QT)
    QT = CHUNK // B         # per-batch owned query range
    QB = min(512, T)        # query block (free dim of logits)
    CT = D // 128           # contraction tiles of the model dim
    NQB = T // QB
    assert T % QB == 0 and D % 128 == 0 and TPB % NCORES == 0
    assert QB % KB == 0 and QB == 2 * QT and NQB * 2 == NCORES
    return B, T, D, H, HD, TPB, CHUNK, QT, QB, CT, NQB


def build_nc(cfg=CFG_FULL):
    """Build + compile the (identical-on-every-core) Bass graph."""
    import concourse.bass as bass
    import concourse.tile as tile
    from concourse import bacc
    import concourse.mybir as mybir

    B, T, D, H, HD, TPB, CHUNK, QT, QB, CT, NQB = _derived(cfg)
    f32 = mybir.dt.float32
    bf16 = mybir.dt.bfloat16
    G = QB // KB  # k-tiles per q-block step

    nc = bacc.Bacc("TRN2", target_bir_lowering=False, debug=False,
                   num_devices=NCORES)

    # ---- kernel I/O ----
    xt = nc.dram_tensor("xt", [D, TPB], bf16, kind="ExternalInput")
    wqkvt = nc.dram_tensor("wqkvt", [D, 3 * 128], bf16, kind="ExternalInput")
    wpt = nc.dram_tensor("wpt", [D, D], bf16, kind="ExternalInput")
    out = nc.dram_tensor("out", [D, CHUNK], bf16, kind="ExternalOutput")

    # AllToAll buffers. Batches 0-1: rows [j*128 + head*64 + d] -> core j,
    # QT cols. Batches 2-3: two half-batch buffers of QT/2 cols each, so
    # the collective chain at the tail is finer-grained.
    y_send = [nc.dram_tensor(f"y_send_{b}", [NCORES * 128, QT], bf16)
              for b in range(2)]
    y_recv = [nc.dram_tensor(f"y_recv_{b}", [NCORES * 128, QT], bf16)
              for b in range(2)]
    y_sendh = {b: [nc.dram_tensor(f"y_sendh{b}_{hh}",
                                  [NCORES * 128, QT // 2], bf16)
                   for hh in range(2)] for b in (2, 3)}
    y_recvh = {b: [nc.dram_tensor(f"y_recvh{b}_{hh}",
                                  [NCORES * 128, QT // 2], bf16)
                   for hh in range(2)] for b in (2, 3)}

    # the one causal triangle every diagonal k-tile needs: m[i,c] = (i <= c)
    tri_np = (np.arange(128)[:, None] <= np.arange(KB)[None, :]) \
        .astype(ml_dtypes.bfloat16)
    tri_dram = nc.inline_tensor(tri_np, name="causal_tri")

    with tile.TileContext(nc) as tc:
        with (
            tc.tile_pool(name="singles", bufs=1) as singles,
            tc.tile_pool(name="xpool", bufs=3) as xpool,
            tc.tile_pool(name="xhpool", bufs=3) as xhpool,
            tc.tile_pool(name="qk_ps", bufs=2, space="PSUM") as qk_ps,
            tc.tile_pool(name="psy_ps", bufs=2, space="PSUM") as psy_ps,
            tc.tile_pool(name="s_ps", bufs=2, space="PSUM") as s_ps,
            tc.tile_pool(name="ppool", bufs=6) as ppool,
            tc.tile_pool(name="npool", bufs=3) as npool,
        ):
            # ---- persistent SBUF; prelude tiles split so the first matmuls
            # depend on single small DMAs, and DMA count stays low (each
            # HWDGE slot is ~0.63us serial) ----
            wq0_sb = singles.tile([128, 1, 3 * 128], bf16)
            wqr = wqkvt.ap().rearrange("(ct p) o -> p ct o", p=128)
            xr = xt.ap().rearrange("(ct p) t -> p ct t", p=128)
            x0_tiles = [singles.tile([128, n, 512], bf16, name=f"x0_{i}")
                        for i, n in enumerate((1, 1, 3, 3))]
            x0_cts = [(0, 1), (1, 2), (2, 5), (5, 8)]
            # issue order = first-consumed order (DMA_ENGINES is serial):
            # x ct0, wq ct0, wq ct1-3, x ct1.., wq ct4-7, x tail, triangle
            nc.scalar.dma_start(out=x0_tiles[0], in_=xr[:, 0:1, 0:512])
            nc.sync.dma_start(out=wq0_sb, in_=wqr[:, 0:1, :])
            wqa_sb = singles.tile([128, 3, 3 * 128], bf16)
            nc.sync.dma_start(out=wqa_sb, in_=wqr[:, 1:4, :])
            nc.scalar.dma_start(out=x0_tiles[1], in_=xr[:, 1:2, 0:512])
            nc.scalar.dma_start(out=x0_tiles[2], in_=xr[:, 2:5, 0:512])
            wqb_sb = singles.tile([128, CT - 4, 3 * 128], bf16)
            nc.sync.dma_start(out=wqb_sb, in_=wqr[:, 4:CT, :])
            nc.scalar.dma_start(out=x0_tiles[3], in_=xr[:, 5:8, 0:512])
            tri_sb = singles.tile([128, KB], bf16)
            nc.sync.dma_start(out=tri_sb, in_=tri_dram.ap())

            def w_ap(ct, sl):
                if ct == 0:
                    return wq0_sb[:, 0, sl]
                if ct < 4:
                    return wqa_sb[:, ct - 1, sl]
                return wqb_sb[:, ct - 4, sl]

            def x0_ap(ct):
                for (c0, c1), t_ in zip(x0_cts, x0_tiles):
                    if c0 <= ct < c1:
                        return t_[:, ct - c0, :]

            wpt_sb = singles.tile([128, CT, D], bf16)

            q_sb = singles.tile([128, TPB], bf16)   # [2*64 qdim, tok]
            k_sb = singles.tile([128, TPB], bf16)   # [2*64 kdim, tok]
            # V natural + ones column (64); cols 65:128 left as garbage —
            # they only feed psy rows 65:127, which are never read
            v_sb = singles.tile([128, 2, TPB // 128, 128], bf16)
            nc.vector.memset(v_sb[:, :, :, 64:65], 1.0)

            def qkv_tt(tt, prelude=False, split=False):
                if prelude:
                    xa = x0_ap
                elif split:
                    # two half-tiles: matmul deps are tile-granular, so the
                    # first ct matmul must not wait for the whole 1MB load
                    xh = [xhpool.tile([128, CT // 2, 512], bf16,
                                      tag=f"xh{i}", name=f"xh{i}_{tt}")
                          for i in range(2)]
                    for i in range(2):
                        nc.scalar.dma_start(
                            out=xh[i],
                            in_=xr[:, i * 4:(i + 1) * 4,
                                   tt * 512:(tt + 1) * 512])
                    xa = lambda ct: xh[ct // 4][:, ct % 4, :]  # noqa: E731
                else:
                    x_sb = xpool.tile([128, CT, 512], bf16, tag="x")
                    nc.sync.dma_start(
                        out=x_sb, in_=xr[:, :, tt * 512:(tt + 1) * 512])
                    xa = lambda ct: x_sb[:, ct, :]  # noqa: E731
                # Q^T and K^T: [2 heads * 64 dims, 512 tokens]
                for u, dst in ((0, q_sb), (1, k_sb)):
                    psqk = qk_ps.tile([128, 512], f32, tag="qk")
                    for ct in range(CT):
                        nc.tensor.matmul(
                            psqk,
                            lhsT=w_ap(ct, slice(u * 128, (u + 1) * 128)),
                            rhs=xa(ct),
                            start=(ct == 0), stop=(ct == CT - 1))
                    nc.vector.tensor_copy(
                        out=dst[:, tt * 512:(tt + 1) * 512], in_=psqk)
                # V natural: [128 tokens, 2 heads * 64 dims]
                for s4 in range(4):
                    t128 = tt * 4 + s4
                    psv = qk_ps.tile([128, 512], f32, tag="qk")
                    pv = psv[:, 0:128]
                    for ct in range(CT):
                        nc.tensor.matmul(
                            pv,
                            lhsT=xa(ct)[:, s4 * 128:(s4 + 1) * 128],
                            rhs=w_ap(ct, slice(256, 384)),
                            start=(ct == 0), stop=(ct == CT - 1))
                    nc.vector.tensor_copy(
                        out=v_sb[:, :, t128, 0:64],
                        in_=pv.rearrange("p (h d) -> p h d", h=2))

            def scatter_y(b, qb, ya):
                """One merged DMA: both heads+dest cores into the A2A buf.

                ya is [64, ncc, 2, tc] (dest-core-major, then head) so the
                source iterates (c, h) with an arithmetic stride and the
                4-dim AP balances down to 3 on both sides.
                """
                if b < 2:
                    dst = y_send[b].ap().rearrange(
                        "(j h p) t -> p j h t", h=2, p=64)
                    nc.sync.dma_start(
                        out=dst[:, 2 * qb:2 * qb + 2, :, :], in_=ya)
                else:
                    half, qq = divmod(qb, 2)
                    dst = y_sendh[b][half].ap().rearrange(
                        "(j h p) t -> p j h t", h=2, p=64)
                    nc.sync.dma_start(
                        out=dst[:, qq * 4:(qq + 1) * 4, :, :], in_=ya)

            def gate_after(write_tile, ya_gate):
                """Force a later instruction that WRITES `write_tile` to be
                scheduled after `ya_gate` is produced: a 1-element gpsimd op
                READS both, creating gate-RAW + writer-WAR edges the list
                scheduler must respect (it otherwise hoists dep-free DMAs to
                where their semaphore wait blocks the whole Pool queue)."""
                def elem0(t):
                    ap = t[tuple(slice(0, 1) for _ in t.shape)]
                    names = " ".join(f"d{i}" for i in range(len(t.shape)))
                    return ap.rearrange(
                        f"d0 {names[3:]} -> d0 ({names[3:]})") \
                        if len(t.shape) > 2 else ap

                scratch = npool.tile([1, 1], bf16, tag="gate")
                nc.gpsimd.tensor_mul(scratch, elem0(write_tile),
                                     elem0(ya_gate))

            def attention_qblock(b, qb):
                t0 = b * T
                q0 = t0 + qb * QB
                n_kk = (qb + 1) * G  # causal k-tiles
                n_g = (n_kk + 1) // 2
                psy = [psy_ps.tile([128, QB], f32, tag="psy",
                                   name=f"psy{h}") for h in range(2)]
                p_tiles = []  # (kks, sts, h, p_sb)

                def av(kks, sts, hh, pp):
                    for u, (kk, st) in enumerate(zip(kks, sts)):
                        nc.tensor.matmul(
                            psy[hh][0:128, st:QB],
                            lhsT=v_sb[:, hh, (t0 // 128) + kk, :],
                            rhs=pp[:, u, st:QB],
                            start=(kk == 0), stop=(kk == n_kk - 1))

                for g in range(n_g):
                    kks = [k for k in (g * 2, g * 2 + 1) if k < n_kk]
                    # per-tile valid column start (diagonal narrowing)
                    dls = [k * KB - qb * QB for k in kks]
                    sts = [max(0, d) for d in dls]
                    gst = min(sts)  # group exp column start
                    for h in range(2):
                        hp = h * 64
                        pss = s_ps.tile([128, 2, QB], f32, tag="s")
                        p_sb = ppool.tile([128, 2, QB], bf16, tag="p")
                        for u, kk in enumerate(kks):
                            k0 = t0 + kk * KB
                            st = sts[u]
                            nc.tensor.matmul(
                                pss[:, u, st:QB],
                                lhsT=k_sb[hp:hp + 64, k0:k0 + KB],
                                rhs=q_sb[hp:hp + 64, q0 + st:q0 + QB],
                                start=True, stop=True)
                        nc.scalar.activation(
                            out=p_sb[:, 0:len(kks), gst:QB],
                            in_=pss[:, 0:len(kks), gst:QB],
                            func=mybir.ActivationFunctionType.Exp,
                            scale=float(HD) ** -0.5)
                        # causal mask: only the KB-wide triangle of each
                        # diagonal tile (AV never reads left of a tile's st)
                        for u in range(len(kks)):
                            dl = dls[u]
                            if dl >= 0:
                                nc.vector.tensor_mul(
                                    p_sb[:, u, dl:dl + KB],
                                    p_sb[:, u, dl:dl + KB], tri_sb)
                        p_tiles.append((kks, sts, h, p_sb))
                        attention_qblock.last_p = p_sb
                        # software-pipeline: AV of group g-1, both heads
                        if g >= 1 and h == 1:
                            for args in p_tiles[-4:-2]:
                                av(*args)
                for args in p_tiles[-2:]:
                    av(*args)

                # normalize: reciprocal of the denominator row (DVE),
                # broadcast across 64 partitions (gpsimd), one DVE multiply
                # per head into the combined bf16 scatter tile. Mid-batch
                # q-blocks first evacuate Y to SBUF so the PSUM banks free
                # early (the next q-block's AV is gated on them); the last
                # q-block skips the evacuation — its latency feeds the A2A.
                ncc = 2 if b < 2 else 4
                tc_ = QB // ncc
                # PSUM-direct normalize (lower latency, later psy release)
                # only where the A2A chain is tail-critical; everything else
                # releases its psy banks early instead
                last = qb == NQB - 1 and b == B - 1
                # where the A2A has slack, push the recip/broadcast/multiply
                # chain down the priority heap so it never delays the next
                # q-block's masks on DVE. b2-qb2/3 feed A2A(2b) whose end
                # drives the tail collective chain — keep those prompt.
                defer = 3000 if (b < 2 or (b == 2 and qb < 2)) else 0
                ya = npool.tile([64, ncc, 2, tc_], bf16, tag="ya")
                ysrcs = []
                for h in range(2):
                    if last:
                        ysrcs.append(psy[h])
                    else:
                        ysrc = npool.tile([65, QB], f32, tag=f"y65{h}")
                        nc.vector.tensor_copy(out=ysrc, in_=psy[h][0:65, :])
                        ysrcs.append(ysrc)
                import contextlib
                if defer:
                    prio = tc.high_priority(offset=-defer)
                elif last and b == B - 1:
                    # the very last normalize feeds the tail A2A: outrank
                    # the gate ops racing for the Pool engine
                    prio = tc.high_priority()
                else:
                    prio = contextlib.nullcontext()
                with prio:
                    for h in range(2):
                        ysrc = ysrcs[h]
                        rf = npool.tile([1, QB], f32, tag=f"rf{h}")
                        nc.vector.reciprocal(out=rf, in_=ysrc[64:65, :])
                        rb = npool.tile([64, QB], f32, tag=f"rb{h}")
                        nc.gpsimd.partition_broadcast(rb, rf)
                        nc.vector.tensor_mul(
                            ya[:, :, h, :],
                            ysrc[0:64, :].rearrange("p (c t) -> p c t",
                                                    c=ncc),
                            rb.rearrange("p (c t) -> p c t", c=ncc))
                    scatter_y(b, qb, ya)
                return ya

            yb_tiles = [singles.tile([128, NCORES, QT], bf16,
                                     name=f"yb_sb{b}") for b in range(B)]

            def proj_obs(b, obs, col0=0, ncols=QT, alt=False):
                """Projection for output blocks `obs`; stores are batched in
                groups of up to 4 contiguous obs (one DMA each) — per-DMA
                issue (~0.6us queue + ~0.6us HWDGE, serialized) otherwise
                throttles the tail."""
                yb_sb = yb_tiles[b]
                obs = list(obs)
                group, g0 = None, None

                def flush():
                    nonlocal group, g0
                    if group is None:
                        return
                    n = group
                    dst = out.ap()[g0 * 128:(g0 + n) * 128,
                                   b * QT + col0:b * QT + col0 + ncols]
                    nc.sync.dma_start(
                        out=dst.rearrange("(o p) t -> p o t", p=128),
                        in_=o4[:, 0:n, :])
                    group = None

                for k, ob in enumerate(obs):
                    if alt and k % 2:  # tail-only: rotate 4 PSUM banks
                        pso = psy_ps.tile([128, QB], f32, tag="psy")
                    else:
                        pso = qk_ps.tile([128, 512], f32, tag="qk")
                    po = pso[:, 0:ncols]
                    for i in range(NCORES):
                        nc.tensor.matmul(
                            po,
                            lhsT=wpt_sb[:, i, ob * 128:(ob + 1) * 128],
                            rhs=yb_sb[:, i, col0:col0 + ncols],
                            start=(i == 0), stop=(i == NCORES - 1))
                    if group is None:
                        o4 = npool.tile([128, 4, ncols], bf16, tag="osb",
                                        name=f"o4_{b}_{ob}_{col0}")
                        group, g0 = 0, ob
                    nc.vector.tensor_copy(out=o4[:, group, :], in_=po)
                    group += 1
                    if group == 4 or k == len(obs) - 1 or obs[k + 1] != ob + 1:
                        flush()

            def a2a(ins_t, outs_t):
                nc.gpsimd.collective_compute(
                    "AllToAll", mybir.AluOpType.bypass,
                    replica_groups=[list(range(NCORES))],
                    ins=[ins_t.ap()], outs=[outs_t.ap()])

            def readback(b, half=None):
                if half is None:
                    nc.gpsimd.dma_start(
                        out=yb_tiles[b],
                        in_=y_recv[b].ap().rearrange("(i p) t -> p i t",
                                                     p=128))
                else:
                    hq = QT // 2
                    src = y_recvh[b][half].ap().rearrange(
                        "(i p) t -> p i t", p=128)
                    if b == 3 and half == 1:
                        # split so proj(3b) can start on the first columns
                        # while the rest still transfers
                        for c0, c1 in ((0, hq // 2), (hq // 2, hq)):
                            nc.gpsimd.dma_start(
                                out=yb_tiles[b][:, :, half * hq + c0:
                                                half * hq + c1],
                                in_=src[:, :, c0:c1])
                    else:
                        nc.gpsimd.dma_start(
                            out=yb_tiles[b][:, :, half * hq:(half + 1) * hq],
                            in_=src)

            # ---- main pipeline ----
            # Causal structure: att(b, qb) only reads K/V token-tiles up to
            # tt = 4b+qb, so QKV is loaded just-in-time, one window ahead —
            # no bulk prelude (startup compresses to one x tile).
            qkv_tt(0, prelude=True)
            W_TTS = {0: [1, 2], 1: [3, 4], 2: [5], 3: [6], 4: [7],
                     5: [8], 6: [9], 7: [10], 8: [11], 9: [12],
                     10: [13], 11: [14], 12: [15]}

            wpr = wpt.ap().rearrange("(ct p) o -> p ct o", p=128)
            for b in range(B):
                for qb in range(NQB):
                    w = b * NQB + qb
                    ya = attention_qblock(b, qb)
                    if b == 0:
                        # wpt fetched in 4 chunks spread over batch-0
                        # attention: off the startup burst, each gated so
                        # the scheduler can't hoist it into the prelude
                        sl = slice(2 * qb, 2 * qb + 2)
                        gate_after(wpt_sb[:, sl, :], ya)
                        nc.gpsimd.dma_start(out=wpt_sb[:, sl, :],
                                            in_=wpr[:, sl, :])
                    if b == 2 and qb == 2:
                        gate_after(yb_tiles[0], ya)
                        readback(0)
                    if b == 3 and qb == 0:
                        gate_after(yb_tiles[1], ya)
                        readback(1)
                        gate_after(yb_tiles[2][:, :, 0:QT // 2], ya)
                        readback(2, half=0)
                    if b == 3 and qb == 2:
                        gate_after(yb_tiles[2][:, :, QT // 2:QT], ya)
                        readback(2, half=1)
                    for tt in W_TTS.get(w, ()):
                        qkv_tt(tt, split=True)
                    if b >= 2 and qb == 1:  # first half-A2A of this batch
                        a2a(y_sendh[b][0], y_recvh[b][0])
                    if b == B - 1 and qb > 0:
                        # batch 3: qkv(15) fills qb0's bubbles; one proj(0)
                        # chunk per later window; the rest covers the tail
                        proj_obs(0, [qb - 1])
                if b < 2:
                    a2a(y_send[b], y_recv[b])
                elif b == 2:
                    a2a(y_sendh[2][1], y_recvh[2][1])

            # tail: second half-A2A of batch 3, covered by proj(1), proj(2)
            # and the first half of proj(3). Remaining readbacks are gated
            # behind the last normalize so they never stall mid-batch
            # broadcasts; their issue cost hides under the collective.
            a2a(y_sendh[3][1], y_recvh[3][1])
            gate_after(yb_tiles[3][:, :, 0:QT // 2], ya)
            readback(3, half=0)
            readback(3, half=1)  # waits on A2A(3b); nothing behind it
            # Hold the deferred proj streams hostage to the last normalize:
            # an exact *1.0 rewrite of one yb element per half, gated on
            # att(3,qb3)'s output, stops the scheduler stealing them for
            # mid-kernel bubbles — they must stay available to cover the
            # tail A2A.
            late_one = npool.tile([1, 1], f32, tag="late1")
            lp = attention_qblock.last_p  # (3,qb3)'s final exp tile: ready
            # col QB-1 is exp-written for every group (gst < QB); col 0 can
            # be stale PSUM garbage (NaN would turn is_ge into 0.0)
            p_e = lp[0:1, 0:1, QB - 1:QB].rearrange("p a b -> p (a b)")
            nc.vector.tensor_tensor(out=late_one, in0=p_e, in1=p_e,
                                    op=mybir.AluOpType.is_ge)
            for bb, c0 in ((0, 0), (1, 0), (2, 0), (2, QT // 2)):
                ybe = yb_tiles[bb][0:1, 0:1, c0:c0 + 1].rearrange(
                    "p a b -> p (a b)")
                nc.gpsimd.tensor_mul(ybe, ybe, late_one)
            proj_obs(0, range(3, D // 128), alt=True)
            proj_obs(1, range(D // 128), alt=True)
            proj_obs(2, range(D // 128), col0=0, ncols=QT // 2, alt=True)
            proj_obs(2, range(D // 128), col0=QT // 2, ncols=QT // 2,
                     alt=True)
            # proj(3) halves depend on late readbacks — keep them at the
            # bottom of the scheduler's priority heap so they can never
            # head-of-line-block the ready proj(0..2) matmuls
            with tc.high_priority(offset=-1_000_000):
                proj_obs(3, range(D // 128), col0=0, ncols=QT // 2,
                         alt=True)
                proj_obs(3, range(D // 128), col0=QT // 2, ncols=QT // 4,
                         alt=True)
                proj_obs(3, range(D // 128), col0=3 * QT // 4,
                         ncols=QT // 4, alt=True)

    nc.compile()
    return nc


def shard_inputs(x, w_qkv, w_proj, cfg=CFG_FULL):
    B, T, D, H, HD, TPB, CHUNK, QT, QB, CT, NQB = _derived(cfg)
    bf16 = ml_dtypes.bfloat16
    xtm = np.ascontiguousarray(
        x.reshape(TPB, D).T).astype(bf16)          # [D, TPB]
    wpt = np.ascontiguousarray(w_proj.T).astype(bf16)  # [D, D]
    in_maps = []
    for i in range(NCORES):
        r = slice(128 * i, 128 * (i + 1))
        wq = w_qkv[0 * D:1 * D][r].T  # [D, 128]
        wk = w_qkv[1 * D:2 * D][r].T
        wv = w_qkv[2 * D:3 * D][r].T
        wqkvt = np.ascontiguousarray(
            np.concatenate([wq, wk, wv], axis=1)).astype(bf16)
        in_maps.append({"xt": xtm, "wqkvt": wqkvt, "wpt": wpt})
    return in_maps


def assemble(outs, cfg=CFG_FULL):
    B, T, D, H, HD, TPB, CHUNK, QT, QB, CT, NQB = _derived(cfg)
    full = np.empty((B, T, D), np.float32)
    ht = T // 2   # 1024
    hq = QT // 2  # 128
    for j in range(NCORES):
        o = np.asarray(outs[j], np.float32)  # [D, B*QT], b-major cols
        for b in range(2):
            full[b, j * QT:(j + 1) * QT, :] = o[:, b * QT:(b + 1) * QT].T
        # batches 2-3: split-half ownership (core j owns 128 tokens/half)
        for b in (2, 3):
            full[b, j * hq:(j + 1) * hq, :] = o[:, b * QT:b * QT + hq].T
            full[b, ht + j * hq:ht + (j + 1) * hq, :] = \
                o[:, b * QT + hq:(b + 1) * QT].T
    return full


_NC_CACHE = None
last_result = None


def kernel(x, w_qkv, w_proj):
    global _NC_CACHE, last_result
    from concourse.bass_utils import run_bass_kernel_spmd

    if _NC_CACHE is None:
        _NC_CACHE = build_nc()
    in_maps = shard_inputs(np.asarray(x, np.float32),
                           np.asarray(w_qkv, np.float32),
                           np.asarray(w_proj, np.float32))
    trace = os.environ.get("BASS_KERNEL_TRACE", "0") == "1"
    res = run_bass_kernel_spmd(_NC_CACHE, in_maps, list(range(NCORES)),
                               trace=trace)
    last_result = res
    outs = [res.results[i]["out"] for i in range(NCORES)]
    return assemble(outs)
